# revision 3
# baseline (speedup 1.0000x reference)
"""Llama4 MoE layer on 8 Trainium2 NeuronCores — expert-parallel routed path,
tensor-parallel shared expert.

Sharding: the router runs on the host while sharding inputs. Core c receives

  - the tokens routed to expert c (pre-scaled by sigmoid(max logit)), padded
    to C1 columns — expert-parallel, no collective (outputs live on disjoint
    token sets, host scatter-adds), plus
  - a 512-wide F-slice (s = c%4) of the SHARED expert applied to half of
    all tokens (block b = c//4, 1024 tokens): tensor-parallel over F with
    G=4. Each core emits a bf16 partial [1024, D]; the host sums the 4
    partials per token block. This cuts the replicated shared weights from
    25MB/core to 6.25MB/core: total HBM traffic ~41.5MB/core vs PE work
    ~176us, so DMA (~290GB/s/core achieved) stays ahead of the PE instead
    of starving it (the baseline moved 54.7MB at 98% DMA occupancy).

Device kernel per core (identical SPMD program): shared wset first (4
f-tiles, 1024 tokens in two 512-col PSUM chunks, chunk-major x packing so
chunk 0 computes while chunk 1 streams), then the expert wset (16 f-tiles,
C1 tokens). Weight tiles stationary as lhsT, tokens stream; silu*up in f32
PSUM -> h bf16 -> down-proj with down tiles stationary, h streaming; y bf16
strips DMA out as each d-pair finishes. PE floor ~= 768 cyc/token *
(C1 + 256-token-equiv shared) ~ 405K cycles ~ 169us at 2.4GHz.
Head: first gu tile split into gate/up halves + x chunk 0 split in d-halves
(all separate tiles — deps are tile-granular) so the first chain starts
~2MB into the stream; 64 densified 256-col warm-up matmuls drive the HAM
clock gate toward 8/8 while the head DMAs land.
"""

import sys

sys.path.insert(0, "/opt/trn_rl_repo")

import ml_dtypes
import numpy as np

import concourse.tile as tile
from concourse import bacc, mybir

T, D, F, E = 2048, 2048, 2048, 8
N_CORES = 8
P = 128
ND, NF = D // P, F // P
G = 4  # F-slices of the shared expert
B = N_CORES // G  # token blocks
C2 = T // B  # shared-expert tokens per core (1024)
NFS = NF // G  # shared f-tiles per core (4)
FS = F // G  # shared F columns per core (512)
QS = 512  # token chunk (PSUM bank width in f32)
f32 = mybir.dt.float32
bf16 = mybir.dt.bfloat16


def build(C1):
    nc = bacc.Bacc(None, target_bir_lowering=False, debug=False)
    # x for shared wset, chunk-major: chunk q holds [P, ND*QS], d-major inside
    xsa = nc.declare_dram_parameter("xsa", [P, ND * C2], bf16, isOutput=False)
    xea = nc.declare_dram_parameter("xea", [P, ND * C1], bf16, isOutput=False)
    wgu = nc.declare_dram_parameter("wgu", [NF, P, 2 * ND * P], bf16, isOutput=False)
    wdp = nc.declare_dram_parameter(
        "wdp", [ND // 2, P, 2 * NF * P], bf16, isOutput=False
    )
    sgu = nc.declare_dram_parameter("sgu", [NFS, P, 2 * ND * P], bf16, isOutput=False)
    sdp = nc.declare_dram_parameter(
        "sdp", [ND // 2, P, 2 * NFS * P], bf16, isOutput=False
    )
    ye = nc.declare_dram_parameter("ye", [P, ND * C1], bf16, isOutput=True)
    ys = nc.declare_dram_parameter("ys", [P, ND * C2], bf16, isOutput=True)

    HALF = ND * QS // 2  # half of one x chunk (d-blocks 0-7 / 8-15)
    with tile.TileContext(nc) as tc:
        with (
            tc.tile_pool(name="xpool", bufs=1) as xp,
            tc.tile_pool(name="wstream", bufs=5) as wp,
            tc.tile_pool(name="hpool", bufs=1) as hp,
            tc.tile_pool(name="work", bufs=2) as sp,
            tc.tile_pool(name="ypool", bufs=3) as yp,
            tc.tile_pool(name="psGU", bufs=2, space="PSUM") as ppG,
            tc.tile_pool(name="psD", bufs=2, space="PSUM") as ppD,
            tc.tile_pool(name="psW", bufs=2, space="PSUM") as ppW,
        ):
            # head: first weight tile as gate/up halves, x chunk 0 in d-halves,
            # interleaved so the first gate chain starts ~2MB into the stream
            gu0g = wp.tile([P, ND * P], bf16, tag="wguh", name="gu0g")
            nc.sync.dma_start(out=gu0g[:], in_=sgu[0, :, : ND * P])
            xq0a = xp.tile([P, HALF], bf16, tag="xq0a", name="xq0a")
            nc.sync.dma_start(out=xq0a[:], in_=xsa[:, :HALF])
            gu0u = wp.tile([P, ND * P], bf16, tag="wguh", name="gu0u")
            nc.sync.dma_start(out=gu0u[:], in_=sgu[0, :, ND * P :])
            xq0b = xp.tile([P, HALF], bf16, tag="xq0b", name="xq0b")
            nc.sync.dma_start(out=xq0b[:], in_=xsa[:, HALF : 2 * HALF])
            xq1 = xp.tile([P, ND * QS], bf16, tag="xq1", name="xq1")
            xe_t = xp.tile([P, ND * C1], bf16, tag="xea", name="xe_t")

            # HAM pre-warm: dense dummy PE activity while the head DMAs land,
            # so the clock gate ramps to 8/8 before real matmuls start
            warm = xp.tile([P, 256], bf16, tag="warm", name="warm")
            nc.vector.memset(warm[:], 0.0)
            for _ in range(64):
                wps = ppW.tile(
                    [P, 256], f32, space="PSUM", tag="warm", bufs=2, name="wps"
                )
                nc.tensor.matmul(
                    out=wps[:], lhsT=warm[:, :P], rhs=warm[:], start=True, stop=True
                )
            # x chunk 1: needed only once chunk 0 is in flight
            nc.sync.dma_start(out=xq1[:], in_=xsa[:, ND * QS :])

            def xcol_shared(d, q0, qw):
                if q0 == 0:
                    t = xq0a if d < ND // 2 else xq0b
                    off = (d % (ND // 2)) * QS
                else:
                    t, off = xq1, d * QS
                return t[:, off : off + qw]

            for w, (gu_p, dp_p, C, nf, y_p) in enumerate(
                [(sgu, sdp, C2, NFS, ys), (wgu, wdp, C1, NF, ye)]
            ):
                chunks = []
                q0 = 0
                while q0 < C:
                    qw = min(QS, C - q0)
                    chunks.append((q0, qw))
                    q0 += qw

                def xcol(d, q0, qw):
                    if w == 0:
                        return xcol_shared(d, q0, qw)
                    return xe_t[:, C * d + q0 : C * d + q0 + qw]

                # ---- gate/up -> h[f] [P, C] bf16 ----
                h_tiles = []
                for f in range(nf):
                    if w == 0 and f == 0:
                        gt, ut = gu0g[:], gu0u[:]
                    else:
                        gu = wp.tile(
                            [P, 2 * ND * P], bf16, tag="wgu", name=f"gu{w}_{f}"
                        )
                        nc.sync.dma_start(out=gu[:], in_=gu_p[f])
                        gt = gu[:, : ND * P]
                        ut = gu[:, ND * P :]
                    if w == 0 and f == 2:
                        # expert-token x, deferred past the startup crunch
                        nc.sync.dma_start(out=xe_t[:], in_=xea[:])
                    h_t = hp.tile(
                        [P, C], bf16, tag=f"h{w}_{f}", bufs=1, name=f"h{w}_{f}"
                    )
                    for q0, qw in chunks:
                        pg = ppG.tile([P, qw], f32, space="PSUM", tag="pg", name="pg")
                        pu = ppG.tile([P, qw], f32, space="PSUM", tag="pu", name="pu")
                        for d in range(ND):
                            nc.tensor.matmul(
                                out=pg[:],
                                lhsT=gt[:, P * d : P * (d + 1)],
                                rhs=xcol(d, q0, qw),
                                start=(d == 0),
                                stop=(d == ND - 1),
                            )
                        for d in range(ND):
                            nc.tensor.matmul(
                                out=pu[:],
                                lhsT=ut[:, P * d : P * (d + 1)],
                                rhs=xcol(d, q0, qw),
                                start=(d == 0),
                                stop=(d == ND - 1),
                            )
                        sig = sp.tile([P, qw], f32, tag="sig", name="sig")
                        nc.scalar.activation(
                            sig[:], pg[:], mybir.ActivationFunctionType.Sigmoid
                        )
                        nc.vector.tensor_tensor(
                            out=sig[:], in0=sig[:], in1=pg[:], op=mybir.AluOpType.mult
                        )
                        nc.vector.tensor_tensor(
                            out=h_t[:, q0 : q0 + qw],
                            in0=sig[:],
                            in1=pu[:],
                            op=mybir.AluOpType.mult,
                        )
                    h_tiles.append(h_t)
                # ---- down-proj, flipped: down tiles stationary, h streams ----
                for j in range(ND // 2):
                    dd = wp.tile(
                        [P, 2 * nf * P], bf16, tag=f"wd{w}", bufs=5, name=f"dd{w}_{j}"
                    )
                    nc.sync.dma_start(out=dd[:], in_=dp_p[j])
                    y_t = yp.tile([P, 2 * C], bf16, tag=f"y{w}", name=f"y{w}_{j}")
                    for half in range(2):
                        dt_ = dd[:, nf * P * half : nf * P * (half + 1)]
                        for q0, qw in chunks:
                            py = ppD.tile(
                                [P, qw], f32, space="PSUM", tag="py", name="py"
                            )
                            for f in range(nf):
                                nc.tensor.matmul(
                                    out=py[:],
                                    lhsT=dt_[:, P * f : P * (f + 1)],
                                    rhs=h_tiles[f][:, q0 : q0 + qw],
                                    start=(f == 0),
                                    stop=(f == nf - 1),
                                )
                            nc.vector.tensor_copy(
                                y_t[:, C * half + q0 : C * half + q0 + qw],
                                py[:],
                            )
                    # stream finished output strips while compute continues
                    nc.sync.dma_start(
                        out=y_p[:, C * 2 * j : C * 2 * (j + 1)], in_=y_t[:]
                    )
    nc.finalize()
    return nc


def _tile_lhsT(w):
    # [A, B] f32 -> [B/P, P, A] bf16 : block b, partition p(a%P), col a_blk*P+q
    A, B = w.shape
    return np.ascontiguousarray(
        w.reshape(A // P, P, B // P, P).transpose(2, 1, 0, 3).reshape(B // P, P, A)
    ).astype(ml_dtypes.bfloat16)


def _fuse_gu(g, u):
    return np.ascontiguousarray(
        np.concatenate([_tile_lhsT(g), _tile_lhsT(u)], axis=2)
    )


def _fuse_dpairs(dw):
    t = _tile_lhsT(dw)
    return np.ascontiguousarray(np.concatenate([t[0::2], t[1::2]], axis=2))


def _pack_x(xc):
    # [C, D] f32 -> [P, ND*C] bf16 with row p holding all d-blocks' row p
    C = xc.shape[0]
    return np.ascontiguousarray(
        xc.T.reshape(ND, P, C).transpose(1, 0, 2).reshape(P, ND * C)
    ).astype(ml_dtypes.bfloat16)


def _pack_x_chunked(xc, qs=QS):
    # chunk-major: concat per-chunk _pack_x along cols
    return np.ascontiguousarray(
        np.concatenate(
            [_pack_x(xc[q : q + qs]) for q in range(0, xc.shape[0], qs)], axis=1
        )
    )


def _unpack_y(ya, C):
    # [P, ND*C] bf16 -> [C, D] f32
    return (
        np.asarray(ya)
        .reshape(P, ND, C)
        .transpose(2, 1, 0)
        .reshape(C, D)
        .astype(np.float32)
    )


def _prep(inputs):
    x = np.asarray(inputs["hidden_states"], dtype=np.float32).reshape(T, D)
    rw = np.asarray(inputs["router_w"], np.float32)

    # router: top-1 expert + sigmoid(max logit) scale, computed while sharding
    logits = x @ rw
    eidx = logits.argmax(-1)
    score = 1.0 / (1.0 + np.exp(-logits.max(-1)))
    xs = x * score[:, None]

    idx = [np.nonzero(eidx == c)[0] for c in range(N_CORES)]
    maxn = max(len(i) for i in idx)
    C1 = max(16, -(-maxn // 16) * 16)

    sg = np.asarray(inputs["shared_gate_w"], np.float32)
    su = np.asarray(inputs["shared_up_w"], np.float32)
    sd = np.asarray(inputs["shared_down_w"], np.float32)
    gw_all = np.asarray(inputs["gate_w"], np.float32)
    uw_all = np.asarray(inputs["up_w"], np.float32)
    dw_all = np.asarray(inputs["down_w"], np.float32)

    in_maps = []
    for c in range(N_CORES):
        b, s = c // G, c % G
        xe = np.zeros((C1, D), np.float32)
        xe[: len(idx[c])] = xs[idx[c]]
        in_maps.append(
            {
                "xsa": _pack_x_chunked(x[C2 * b : C2 * (b + 1)]),
                "xea": _pack_x(xe),
                "wgu": _fuse_gu(gw_all[c], uw_all[c]),
                "wdp": _fuse_dpairs(dw_all[c]),
                "sgu": _fuse_gu(
                    sg[:, FS * s : FS * (s + 1)], su[:, FS * s : FS * (s + 1)]
                ),
                "sdp": _fuse_dpairs(sd[FS * s : FS * (s + 1)]),
            }
        )
    return in_maps, idx, C1


def run(inputs, trace=False, tmpdir=None):
    from concourse.bass_utils import run_bass_kernel_spmd

    in_maps, idx, C1 = _prep(inputs)
    nc = build(C1)
    res = run_bass_kernel_spmd(
        nc, in_maps, core_ids=list(range(N_CORES)), trace=trace, tmpdir=tmpdir
    )
    out = np.zeros((T, D), np.float32)
    for c in range(N_CORES):
        b = c // G
        ye = _unpack_y(res.results[c]["ye"], C1)
        out[idx[c]] += ye[: len(idx[c])]
        out[C2 * b : C2 * (b + 1)] += _unpack_y(res.results[c]["ys"], C2)
    return out.reshape(T // 2, 2, D), res


def kernel(**inputs) -> np.ndarray:
    out, _ = run(inputs)
    return out
